# revision 4
# baseline (speedup 1.0000x reference)
"""Data-parallel Trainium kernel for nn_ActiveSensingFramework.

Strategy: pure data parallel over 8 NeuronCores (bsz 1024 -> 8 x 128).
The forward pass is rewritten in real arithmetic (re/im pairs, no complex
dtype, no slogdet -- unrolled 4x4 Cholesky) so it lowers cleanly through
neuronx-cc, and executed with jax.pmap on the 8 cores. Parameters are
replicated once; only the scalar per-stage logdet sums are reduced on host.
"""

import math
from functools import partial

import numpy as np
import jax
import jax.numpy as jnp

HSZ, NSTAGES, NS, NA, NB, NRF, BSZ = 512, 8, 4, 64, 64, 8, 1024
NCORES = 8
B = BSZ // NCORES

# ---------------- complex helpers on (re, im) tuples ----------------


def _cmatmul(a, b):
    ar, ai = a
    br, bi = b
    return ar @ br - ai @ bi, ar @ bi + ai @ br


def _cT(a):  # conj-transpose of last two dims
    ar, ai = a
    return jnp.swapaxes(ar, -1, -2), -jnp.swapaxes(ai, -1, -2)


def _cadd(a, b):
    return a[0] + b[0], a[1] + b[1]


def _unitmod(a, scale):
    # a / (scale * |a|)
    ar, ai = a
    inv = 1.0 / (scale * jnp.sqrt(ar * ar + ai * ai))
    return ar * inv, ai * inv


def _colnorm(a, axis):
    ar, ai = a
    inv = 1.0 / jnp.sqrt(jnp.sum(ar * ar + ai * ai, axis=axis, keepdims=True))
    return ar * inv, ai * inv


def _ri(y):
    # (B, n, ns) complex pair -> (ns, B, 2n) interleaved re/im
    yr = jnp.moveaxis(y[0], 2, 0)
    yi = jnp.moveaxis(y[1], 2, 0)
    s = jnp.stack([yr, yi], axis=-1)  # (ns, B, n, 2)
    return s.reshape(s.shape[0], s.shape[1], -1)


def _mlp_w(x, p):
    # x: (NS, B, HSZ) -> complex (B, NRF, NS)
    o = jax.nn.relu(x @ p['w1'] + p['b1']) @ p['w2'] + p['b2']
    re = jnp.moveaxis(o[..., :NRF], 0, -1)      # (B, NRF, NS)
    im = jnp.moveaxis(o[..., NRF:2 * NRF], 0, -1)
    return re, im


def _mlp_f(x, p, n):
    # x: (B, 512) -> complex (B, n, NRF)
    o = jax.nn.relu(x @ p['w1'] + p['b1']) @ p['w2'] + p['b2']
    m = n * NRF
    re = o[..., :m].reshape(x.shape[0], n, NRF)
    im = o[..., m:2 * m].reshape(x.shape[0], n, NRF)
    return re, im


def _gru(x, h, p):
    gi = x @ p['wih'] + p['bih']
    gh = h @ p['whh'] + p['bhh']
    ir, iz, inn = jnp.split(gi, 3, axis=-1)
    hr, hz, hn = jnp.split(gh, 3, axis=-1)
    r = jax.nn.sigmoid(ir + hr)
    z = jax.nn.sigmoid(iz + hz)
    n = jnp.tanh(inn + r * hn)
    return (1.0 - z) * n + z * h


def _orthogonal(x):
    # x: complex pair, (B, n, NS). Projections onto ORIGINAL columns.
    xr, xi = x
    orig = [(xr[:, :, k], xi[:, :, k]) for k in range(NS)]
    den = [jnp.sum(o[0] * o[0] + o[1] * o[1], axis=1, keepdims=True)
           for o in orig]
    out = []
    for ii in range(NS):
        vr, vi = orig[ii]
        for jj in range(ii):
            ur, ui = orig[jj]
            nr = jnp.sum(ur * vr + ui * vi, axis=1, keepdims=True)
            ni = jnp.sum(ur * vi - ui * vr, axis=1, keepdims=True)
            cr = nr / den[jj]
            ci = ni / den[jj]
            vr = vr - (cr * ur - ci * ui)
            vi = vi - (cr * ui + ci * ur)
        inv = 1.0 / jnp.sqrt(jnp.sum(vr * vr + vi * vi, axis=1, keepdims=True))
        out.append((vr * inv, vi * inv))
    return (jnp.stack([o[0] for o in out], axis=2),
            jnp.stack([o[1] for o in out], axis=2))


def _logdet_hpd4(Mr, Mi):
    # logdet of Hermitian PD 4x4 batch via unrolled Cholesky. Returns (B,).
    L = [[None] * NS for _ in range(NS)]
    ld = 0.0
    for k in range(NS):
        d = Mr[:, k, k]
        for j in range(k):
            lr, li = L[k][j]
            d = d - (lr * lr + li * li)
        ld = ld + jnp.log(d)
        inv = 1.0 / jnp.sqrt(d)
        for i2 in range(k + 1, NS):
            pr = Mr[:, i2, k]
            pi = Mi[:, i2, k]
            for j in range(k):
                ar, ai = L[i2][j]
                br, bi = L[k][j]
                pr = pr - (ar * br + ai * bi)
                pi = pi - (ai * br - ar * bi)
            L[i2][k] = (pr * inv, pi * inv)
    return ld


# ---------------- per-stage computation (runs on one core's shard) ------


def _side_common(y, hs, gru_p, lin_p, mlp_w_p):
    x = _ri(y)
    hs = _gru(x, hs, gru_p)
    hlin = hs @ lin_p['w'] + lin_p['b']          # (NS, B, 128)
    w0 = _mlp_w(hs, mlp_w_p)                     # (B, NRF, NS)
    flat = jnp.moveaxis(hlin, 0, -1).reshape(hlin.shape[1], 128 * NS)
    return hs, w0, flat


def _stage(p, Hr, Hi, nbr, nbi, nar, nai, hs_a, hs_b,
           Far, Fai, Fbr, Fbi, FWar, FWai):
    H = (Hr, Hi)
    nstd = p['nstd']
    # ---- A transmits to B ----
    ybf = _cmatmul(H, (FWar, FWai))
    ybf = (ybf[0] + nbr * nstd, ybf[1] + nbi * nstd)
    yb = _cmatmul(_cT((Fbr, Fbi)), ybf)          # (B, NRF, NS)
    hs_b, Wb0, flat_b = _side_common(yb, hs_b, p['gru_b'], p['lin_b'],
                                     p['mlp_b_W'])
    Wb = _orthogonal(_cadd(Wb0, yb))
    Fbt = _unitmod(_mlp_f(flat_b, p['mlp_b_Ft'], NB), math.sqrt(NB))
    FWb = _colnorm(_cmatmul(Fbt, Wb), axis=1)
    Fb = _unitmod(_mlp_f(flat_b, p['mlp_b_Fr'], NB), math.sqrt(NB))
    # ---- B transmits to A ----
    yaf = _cmatmul(_cT(H), FWb)
    yaf = (yaf[0] + nar * nstd, yaf[1] + nai * nstd)
    ya = _cmatmul(_cT((Far, Fai)), yaf)
    hs_a, Wa0, flat_a = _side_common(ya, hs_a, p['gru_a'], p['lin_a'],
                                     p['mlp_a_W'])
    Wan = _orthogonal(_cadd(Wa0, ya))
    Fat = _unitmod(_mlp_f(flat_a, p['mlp_a_Ft'], NA), math.sqrt(NA))
    FWa = _colnorm(_cmatmul(Fat, Wan), axis=1)
    Fa = _unitmod(_mlp_f(flat_a, p['mlp_a_Fr'], NA), math.sqrt(NA))
    # ---- final beamformers ----
    Wb0f = _mlp_w(hs_b, p['mlp_b_final'])
    Wa0f = _mlp_w(hs_a, p['mlp_a_final'])
    Wbf = _colnorm(_cmatmul(Fb, _orthogonal(_cadd(Wb0f, yb))), axis=1)
    Waf = _colnorm(_cmatmul(Fa, _orthogonal(_cadd(Wa0f, ya))), axis=1)
    # ---- utility ----
    G = _cmatmul(_cmatmul(_cT(Wbf), H), Waf)     # (B, NS, NS)
    GH = _cT(G)
    Mr = G[0] @ GH[0] - G[1] @ GH[1] + jnp.eye(NS, dtype=jnp.float32)
    Mi = G[0] @ GH[1] + G[1] @ GH[0]
    ld_sum = jnp.sum(_logdet_hpd4(Mr, Mi))
    return (hs_a, hs_b, Fa[0], Fa[1], Fb[0], Fb[1], FWa[0], FWa[1],
            Waf[0], Waf[1], Wbf[0], Wbf[1], ld_sum)


_PMAP_CACHE = {}


def _accel_devices():
    for plat in ('neuron', 'axon'):
        try:
            d = jax.devices(plat)
            if len(d) >= NCORES:
                return d[:NCORES]
        except RuntimeError:
            continue
    return jax.devices()[:NCORES]


def _get_stage_pmap(devs):
    key = tuple(id(d) for d in devs)
    if key not in _PMAP_CACHE:
        _PMAP_CACHE[key] = jax.pmap(_stage, axis_name='c', devices=devs)
    return _PMAP_CACHE[key]


def _np_leaves(d):
    if isinstance(d, dict):
        return {k: _np_leaves(v) for k, v in d.items()}
    return np.asarray(d)


def kernel(channel_re, channel_im, sigma2, noise_a_re, noise_a_im,
           noise_b_re, noise_b_im, params):
    params = _np_leaves(params)
    channel_re = np.asarray(channel_re)
    channel_im = np.asarray(channel_im)
    sigma2 = np.asarray(sigma2)

    nstd = float(np.sqrt(sigma2[0] / 2.0))

    # ---- host-side init (tiny, unbatched) ----
    def unitmod_np(re, im, scale):
        a = scale * np.sqrt(re * re + im * im)
        return (re / a).astype(np.float32), (im / a).astype(np.float32)

    Fbr0, Fbi0 = unitmod_np(np.asarray(params['init_Fb_re']),
                            np.asarray(params['init_Fb_im']), math.sqrt(NB))
    Far0, Fai0 = unitmod_np(np.asarray(params['init_Fa_re']),
                            np.asarray(params['init_Fa_im']), math.sqrt(NA))
    War, Wai = np.asarray(params['init_Wa_re']), np.asarray(params['init_Wa_im'])
    FWar0 = Far0 @ War - Fai0 @ Wai
    FWai0 = Far0 @ Wai + Fai0 @ War
    nrm = np.sqrt((FWar0 * FWar0 + FWai0 * FWai0).sum(axis=0, keepdims=True))
    FWar0 = (FWar0 / nrm).astype(np.float32)
    FWai0 = (FWai0 / nrm).astype(np.float32)

    # ---- shard data: (NCORES, B, ...) ----
    def shard(x):
        x = np.asarray(x)
        return x.reshape((NCORES, B) + x.shape[1:])

    Hr = shard(channel_re)
    Hi = shard(channel_im)
    # noise: (NSTAGES, BSZ, n, NS) -> per-stage (NCORES, B, n, NS)
    nb_r = [shard(np.asarray(noise_b_re)[i]) for i in range(NSTAGES)]
    nb_i = [shard(np.asarray(noise_b_im)[i]) for i in range(NSTAGES)]
    na_r = [shard(np.asarray(noise_a_re)[i]) for i in range(NSTAGES)]
    na_i = [shard(np.asarray(noise_a_im)[i]) for i in range(NSTAGES)]

    # ---- replicated params ----
    prep = {
        'gru_a': params['gru_a'], 'gru_b': params['gru_b'],
        'mlp_a_W': params['mlp_a_W'], 'mlp_b_W': params['mlp_b_W'],
        'mlp_a_final': params['mlp_a_final'],
        'mlp_b_final': params['mlp_b_final'],
        'mlp_a_Fr': params['mlp_a_Fr'], 'mlp_b_Fr': params['mlp_b_Fr'],
        'mlp_a_Ft': params['mlp_a_Ft'], 'mlp_b_Ft': params['mlp_b_Ft'],
        'lin_a': params['lin_a'], 'lin_b': params['lin_b'],
        'nstd': np.float32(nstd),
    }
    devs = _accel_devices()
    stage_pmap = _get_stage_pmap(devs)
    prep = jax.device_put_replicated(prep, devs)

    # ---- initial carry ----
    ones_h = np.ones((NCORES, NS, B, HSZ), np.float32)
    zeros_like = lambda a: np.broadcast_to(a, (NCORES, B) + a.shape).copy()
    carry = dict(
        hs_a=ones_h, hs_b=ones_h.copy(),
        Far=zeros_like(Far0), Fai=zeros_like(Fai0),
        Fbr=zeros_like(Fbr0), Fbi=zeros_like(Fbi0),
        FWar=zeros_like(FWar0), FWai=zeros_like(FWai0),
    )

    ld_total = 0.0
    Waf = Wbf = None
    for i in range(NSTAGES):
        (hs_a, hs_b, Far, Fai, Fbr, Fbi, FWar, FWai,
         Wafr, Wafi, Wbfr, Wbfi, ld) = stage_pmap(
            prep, Hr, Hi, nb_r[i], nb_i[i], na_r[i], na_i[i],
            carry['hs_a'], carry['hs_b'], carry['Far'], carry['Fai'],
            carry['Fbr'], carry['Fbi'], carry['FWar'], carry['FWai'])
        carry = dict(hs_a=hs_a, hs_b=hs_b, Far=Far, Fai=Fai,
                     Fbr=Fbr, Fbi=Fbi, FWar=FWar, FWai=FWai)
        ld_total = ld_total + np.sum(np.asarray(ld), dtype=np.float64)
        Waf = (Wafr, Wafi)
        Wbf = (Wbfr, Wbfi)

    Wa_final = (np.asarray(Waf[0]) + 1j * np.asarray(Waf[1])).astype(
        np.complex64).reshape(BSZ, NA, NS)
    Wb_final = (np.asarray(Wbf[0]) + 1j * np.asarray(Wbf[1])).astype(
        np.complex64).reshape(BSZ, NB, NS)
    loss = np.float32(-ld_total / BSZ)
    return Wa_final, Wb_final, loss


# revision 11
# speedup vs baseline: 3.2965x; 3.2965x over previous
"""Data-parallel Trainium kernel for nn_ActiveSensingFramework.

Strategy: pure data parallel over 8 NeuronCores (bsz 1024 -> 8 x 128).
The forward pass is rewritten in real arithmetic (re/im pairs, no complex
dtype, no slogdet -- unrolled 4x4 Cholesky) so it lowers cleanly through
neuronx-cc, and executed with jax.pmap on the 8 cores. Parameters are
replicated once; only the scalar per-stage logdet sums are reduced on host.
"""

import math
from functools import partial

import numpy as np
import jax
import jax.numpy as jnp

HSZ, NSTAGES, NS, NA, NB, NRF, BSZ = 512, 8, 4, 64, 64, 8, 1024
NCORES = 8
B = BSZ // NCORES

# ---------------- complex helpers on (re, im) tuples ----------------


def _cmatmul(a, b):
    ar, ai = a
    br, bi = b
    return ar @ br - ai @ bi, ar @ bi + ai @ br


def _cT(a):  # conj-transpose of last two dims
    ar, ai = a
    return jnp.swapaxes(ar, -1, -2), -jnp.swapaxes(ai, -1, -2)


def _cadd(a, b):
    return a[0] + b[0], a[1] + b[1]


def _unitmod(a, scale):
    # a / (scale * |a|)
    ar, ai = a
    inv = 1.0 / (scale * jnp.sqrt(ar * ar + ai * ai))
    return ar * inv, ai * inv


def _colnorm(a, axis):
    ar, ai = a
    inv = 1.0 / jnp.sqrt(jnp.sum(ar * ar + ai * ai, axis=axis, keepdims=True))
    return ar * inv, ai * inv


def _ri(y):
    # (B, n, ns) complex pair -> (ns, B, 2n) interleaved re/im
    yr = jnp.moveaxis(y[0], 2, 0)
    yi = jnp.moveaxis(y[1], 2, 0)
    s = jnp.stack([yr, yi], axis=-1)  # (ns, B, n, 2)
    return s.reshape(s.shape[0], s.shape[1], -1)


def _mlp_w(x, p):
    # x: (NS, B, HSZ) -> complex (B, NRF, NS)
    o = jax.nn.relu(x @ p['w1'] + p['b1']) @ p['w2'] + p['b2']
    re = jnp.moveaxis(o[..., :NRF], 0, -1)      # (B, NRF, NS)
    im = jnp.moveaxis(o[..., NRF:2 * NRF], 0, -1)
    return re, im


def _mlp_f(x, p, n):
    # x: (B, 512) -> complex (B, n, NRF)
    o = jax.nn.relu(x @ p['w1'] + p['b1']) @ p['w2'] + p['b2']
    m = n * NRF
    re = o[..., :m].reshape(x.shape[0], n, NRF)
    im = o[..., m:2 * m].reshape(x.shape[0], n, NRF)
    return re, im


def _gru(x, h, p):
    gi = x @ p['wih'] + p['bih']
    gh = h @ p['whh'] + p['bhh']
    ir, iz, inn = jnp.split(gi, 3, axis=-1)
    hr, hz, hn = jnp.split(gh, 3, axis=-1)
    r = jax.nn.sigmoid(ir + hr)
    z = jax.nn.sigmoid(iz + hz)
    n = jnp.tanh(inn + r * hn)
    return (1.0 - z) * n + z * h


def _orthogonal(x):
    # x: complex pair, (B, n, NS). Projections onto ORIGINAL columns.
    xr, xi = x
    orig = [(xr[:, :, k], xi[:, :, k]) for k in range(NS)]
    den = [jnp.sum(o[0] * o[0] + o[1] * o[1], axis=1, keepdims=True)
           for o in orig]
    out = []
    for ii in range(NS):
        vr, vi = orig[ii]
        for jj in range(ii):
            ur, ui = orig[jj]
            nr = jnp.sum(ur * vr + ui * vi, axis=1, keepdims=True)
            ni = jnp.sum(ur * vi - ui * vr, axis=1, keepdims=True)
            cr = nr / den[jj]
            ci = ni / den[jj]
            vr = vr - (cr * ur - ci * ui)
            vi = vi - (cr * ui + ci * ur)
        inv = 1.0 / jnp.sqrt(jnp.sum(vr * vr + vi * vi, axis=1, keepdims=True))
        out.append((vr * inv, vi * inv))
    return (jnp.stack([o[0] for o in out], axis=2),
            jnp.stack([o[1] for o in out], axis=2))


def _logdet_hpd4(Mr, Mi):
    # logdet of Hermitian PD 4x4 batch via unrolled Cholesky. Returns (B,).
    L = [[None] * NS for _ in range(NS)]
    ld = 0.0
    for k in range(NS):
        d = Mr[:, k, k]
        for j in range(k):
            lr, li = L[k][j]
            d = d - (lr * lr + li * li)
        ld = ld + jnp.log(d)
        inv = 1.0 / jnp.sqrt(d)
        for i2 in range(k + 1, NS):
            pr = Mr[:, i2, k]
            pi = Mi[:, i2, k]
            for j in range(k):
                ar, ai = L[i2][j]
                br, bi = L[k][j]
                pr = pr - (ar * br + ai * bi)
                pi = pi - (ai * br - ar * bi)
            L[i2][k] = (pr * inv, pi * inv)
    return ld


# ---------------- per-stage computation (runs on one core's shard) ------


def _side_common(y, hs, gru_p, lin_p, mlp_w_p):
    x = _ri(y)
    hs = _gru(x, hs, gru_p)
    hlin = hs @ lin_p['w'] + lin_p['b']          # (NS, B, 128)
    w0 = _mlp_w(hs, mlp_w_p)                     # (B, NRF, NS)
    flat = jnp.moveaxis(hlin, 0, -1).reshape(hlin.shape[1], 128 * NS)
    return hs, w0, flat


def _stage(p, Hr, Hi, nbr, nbi, nar, nai, hs_a, hs_b,
           Far, Fai, Fbr, Fbi, FWar, FWai, ld_acc):
    H = (Hr, Hi)
    nstd = p['nstd']
    # ---- A transmits to B ----
    ybf = _cmatmul(H, (FWar, FWai))
    ybf = (ybf[0] + nbr * nstd, ybf[1] + nbi * nstd)
    yb = _cmatmul(_cT((Fbr, Fbi)), ybf)          # (B, NRF, NS)
    hs_b, Wb0, flat_b = _side_common(yb, hs_b, p['gru_b'], p['lin_b'],
                                     p['mlp_b_W'])
    Wb = _orthogonal(_cadd(Wb0, yb))
    Fbt = _unitmod(_mlp_f(flat_b, p['mlp_b_Ft'], NB), math.sqrt(NB))
    FWb = _colnorm(_cmatmul(Fbt, Wb), axis=1)
    Fb = _unitmod(_mlp_f(flat_b, p['mlp_b_Fr'], NB), math.sqrt(NB))
    # ---- B transmits to A ----
    yaf = _cmatmul(_cT(H), FWb)
    yaf = (yaf[0] + nar * nstd, yaf[1] + nai * nstd)
    ya = _cmatmul(_cT((Far, Fai)), yaf)
    hs_a, Wa0, flat_a = _side_common(ya, hs_a, p['gru_a'], p['lin_a'],
                                     p['mlp_a_W'])
    Wan = _orthogonal(_cadd(Wa0, ya))
    Fat = _unitmod(_mlp_f(flat_a, p['mlp_a_Ft'], NA), math.sqrt(NA))
    FWa = _colnorm(_cmatmul(Fat, Wan), axis=1)
    Fa = _unitmod(_mlp_f(flat_a, p['mlp_a_Fr'], NA), math.sqrt(NA))
    # ---- final beamformers ----
    Wb0f = _mlp_w(hs_b, p['mlp_b_final'])
    Wa0f = _mlp_w(hs_a, p['mlp_a_final'])
    Wbf = _colnorm(_cmatmul(Fb, _orthogonal(_cadd(Wb0f, yb))), axis=1)
    Waf = _colnorm(_cmatmul(Fa, _orthogonal(_cadd(Wa0f, ya))), axis=1)
    # ---- utility ----
    G = _cmatmul(_cmatmul(_cT(Wbf), H), Waf)     # (B, NS, NS)
    GH = _cT(G)
    Mr = G[0] @ GH[0] - G[1] @ GH[1] + jnp.eye(NS, dtype=jnp.float32)
    Mi = G[0] @ GH[1] + G[1] @ GH[0]
    ld_sum = ld_acc + jnp.sum(_logdet_hpd4(Mr, Mi))
    return (hs_a, hs_b, Fa[0], Fa[1], Fb[0], Fb[1], FWa[0], FWa[1],
            Waf[0], Waf[1], Wbf[0], Wbf[1], ld_sum)


_PMAP_CACHE = {}


def _accel_devices():
    for plat in ('neuron', 'axon'):
        try:
            d = jax.devices(plat)
            if len(d) >= NCORES:
                return d[:NCORES]
        except RuntimeError:
            continue
    return jax.devices()[:NCORES]


def _get_stage_pmap(devs):
    key = tuple(id(d) for d in devs)
    if key not in _PMAP_CACHE:
        _PMAP_CACHE[key] = jax.pmap(_stage, axis_name='c', devices=devs)
    return _PMAP_CACHE[key]


def _np_leaves(d):
    if isinstance(d, dict):
        return {k: _np_leaves(v) for k, v in d.items()}
    return np.asarray(d)


def kernel(channel_re, channel_im, sigma2, noise_a_re, noise_a_im,
           noise_b_re, noise_b_im, params):
    params = _np_leaves(params)
    channel_re = np.asarray(channel_re)
    channel_im = np.asarray(channel_im)
    sigma2 = np.asarray(sigma2)

    nstd = float(np.sqrt(sigma2[0] / 2.0))

    # ---- host-side init (tiny, unbatched) ----
    def unitmod_np(re, im, scale):
        a = scale * np.sqrt(re * re + im * im)
        return (re / a).astype(np.float32), (im / a).astype(np.float32)

    Fbr0, Fbi0 = unitmod_np(np.asarray(params['init_Fb_re']),
                            np.asarray(params['init_Fb_im']), math.sqrt(NB))
    Far0, Fai0 = unitmod_np(np.asarray(params['init_Fa_re']),
                            np.asarray(params['init_Fa_im']), math.sqrt(NA))
    War, Wai = np.asarray(params['init_Wa_re']), np.asarray(params['init_Wa_im'])
    FWar0 = Far0 @ War - Fai0 @ Wai
    FWai0 = Far0 @ Wai + Fai0 @ War
    nrm = np.sqrt((FWar0 * FWar0 + FWai0 * FWai0).sum(axis=0, keepdims=True))
    FWar0 = (FWar0 / nrm).astype(np.float32)
    FWai0 = (FWai0 / nrm).astype(np.float32)

    # ---- shard data: (NCORES, B, ...) ----
    def shard(x):
        x = np.asarray(x)
        return x.reshape((NCORES, B) + x.shape[1:])

    devs = _accel_devices()
    stage_pmap = _get_stage_pmap(devs)

    def dshard(x):
        x = shard(x)
        return jax.device_put_sharded([x[i] for i in range(NCORES)], devs)

    Hr = dshard(channel_re)
    Hi = dshard(channel_im)
    # noise: (NSTAGES, BSZ, n, NS) -> per-stage (NCORES, B, n, NS), staged
    # onto the devices up front so the stage loop does no H2D transfers.
    nb_r = [dshard(np.asarray(noise_b_re)[i]) for i in range(NSTAGES)]
    nb_i = [dshard(np.asarray(noise_b_im)[i]) for i in range(NSTAGES)]
    na_r = [dshard(np.asarray(noise_a_re)[i]) for i in range(NSTAGES)]
    na_i = [dshard(np.asarray(noise_a_im)[i]) for i in range(NSTAGES)]

    # ---- replicated params ----
    prep = {
        'gru_a': params['gru_a'], 'gru_b': params['gru_b'],
        'mlp_a_W': params['mlp_a_W'], 'mlp_b_W': params['mlp_b_W'],
        'mlp_a_final': params['mlp_a_final'],
        'mlp_b_final': params['mlp_b_final'],
        'mlp_a_Fr': params['mlp_a_Fr'], 'mlp_b_Fr': params['mlp_b_Fr'],
        'mlp_a_Ft': params['mlp_a_Ft'], 'mlp_b_Ft': params['mlp_b_Ft'],
        'lin_a': params['lin_a'], 'lin_b': params['lin_b'],
        'nstd': np.float32(nstd),
    }
    prep = jax.device_put_replicated(prep, devs)

    # ---- initial carry ----
    def drepl(a):  # replicate a small per-shard array to all cores
        return jax.device_put_sharded(
            [np.broadcast_to(a, (B,) + a.shape).copy() for _ in range(NCORES)],
            devs)

    ones_h = np.ones((NS, B, HSZ), np.float32)
    carry = [
        jax.device_put_sharded([ones_h] * NCORES, devs),          # hs_a
        jax.device_put_sharded([ones_h.copy()] * NCORES, devs),   # hs_b
        drepl(Far0), drepl(Fai0), drepl(Fbr0), drepl(Fbi0),
        drepl(FWar0), drepl(FWai0),
        jax.device_put_sharded([np.zeros((), np.float32)] * NCORES, devs),
    ]

    Waf = Wbf = None
    for i in range(NSTAGES):
        out = stage_pmap(prep, Hr, Hi, nb_r[i], nb_i[i], na_r[i], na_i[i],
                         *carry)
        (hs_a, hs_b, Far, Fai, Fbr, Fbi, FWar, FWai,
         Wafr, Wafi, Wbfr, Wbfi, ld) = out
        carry = [hs_a, hs_b, Far, Fai, Fbr, Fbi, FWar, FWai, ld]
        Waf = (Wafr, Wafi)
        Wbf = (Wbfr, Wbfi)

    ld_total = np.sum(np.asarray(carry[-1]), dtype=np.float64)
    Wa_final = (np.asarray(Waf[0]) + 1j * np.asarray(Waf[1])).astype(
        np.complex64).reshape(BSZ, NA, NS)
    Wb_final = (np.asarray(Wbf[0]) + 1j * np.asarray(Wbf[1])).astype(
        np.complex64).reshape(BSZ, NB, NS)
    loss = np.float32(-ld_total / BSZ)
    return Wa_final, Wb_final, loss


# revision 15
# speedup vs baseline: 3.8234x; 1.1598x over previous
"""Data-parallel Trainium kernel for nn_ActiveSensingFramework.

Strategy: pure data parallel over 8 NeuronCores (bsz 1024 -> 8 x 128).
The forward pass is rewritten in real arithmetic (re/im pairs, no complex
dtype, no slogdet -- unrolled 4x4 Cholesky) so it lowers cleanly through
neuronx-cc, and executed with jax.pmap on the 8 cores. Parameters are
replicated once; only the scalar per-stage logdet sums are reduced on host.
"""

import math
from functools import partial

import numpy as np
import jax
import jax.numpy as jnp

HSZ, NSTAGES, NS, NA, NB, NRF, BSZ = 512, 8, 4, 64, 64, 8, 1024
NCORES = 8
B = BSZ // NCORES

# ---------------- complex helpers on (re, im) tuples ----------------


def _cmatmul(a, b):
    ar, ai = a
    br, bi = b
    return ar @ br - ai @ bi, ar @ bi + ai @ br


def _cT(a):  # conj-transpose of last two dims
    ar, ai = a
    return jnp.swapaxes(ar, -1, -2), -jnp.swapaxes(ai, -1, -2)


def _cadd(a, b):
    return a[0] + b[0], a[1] + b[1]


def _unitmod(a, scale):
    # a / (scale * |a|)
    ar, ai = a
    inv = 1.0 / (scale * jnp.sqrt(ar * ar + ai * ai))
    return ar * inv, ai * inv


def _colnorm(a, axis):
    ar, ai = a
    inv = 1.0 / jnp.sqrt(jnp.sum(ar * ar + ai * ai, axis=axis, keepdims=True))
    return ar * inv, ai * inv


def _ri(y):
    # (B, n, ns) complex pair -> (ns, B, 2n) interleaved re/im
    yr = jnp.moveaxis(y[0], 2, 0)
    yi = jnp.moveaxis(y[1], 2, 0)
    s = jnp.stack([yr, yi], axis=-1)  # (ns, B, n, 2)
    return s.reshape(s.shape[0], s.shape[1], -1)


def _mlp_w(x, p):
    # x: (NS, B, HSZ) -> complex (B, NRF, NS)
    o = jax.nn.relu(x @ p['w1'] + p['b1']) @ p['w2'] + p['b2']
    re = jnp.moveaxis(o[..., :NRF], 0, -1)      # (B, NRF, NS)
    im = jnp.moveaxis(o[..., NRF:2 * NRF], 0, -1)
    return re, im


def _mlp_f(x, p, n):
    # x: (B, 512) -> complex (B, n, NRF)
    o = jax.nn.relu(x @ p['w1'] + p['b1']) @ p['w2'] + p['b2']
    m = n * NRF
    re = o[..., :m].reshape(x.shape[0], n, NRF)
    im = o[..., m:2 * m].reshape(x.shape[0], n, NRF)
    return re, im


def _gru(x, h, p):
    gi = x @ p['wih'] + p['bih']
    gh = h @ p['whh'] + p['bhh']
    ir, iz, inn = jnp.split(gi, 3, axis=-1)
    hr, hz, hn = jnp.split(gh, 3, axis=-1)
    r = jax.nn.sigmoid(ir + hr)
    z = jax.nn.sigmoid(iz + hz)
    n = jnp.tanh(inn + r * hn)
    return (1.0 - z) * n + z * h


def _orthogonal(x):
    # x: complex pair, (B, n, NS). Projections onto ORIGINAL columns.
    xr, xi = x
    orig = [(xr[:, :, k], xi[:, :, k]) for k in range(NS)]
    den = [jnp.sum(o[0] * o[0] + o[1] * o[1], axis=1, keepdims=True)
           for o in orig]
    out = []
    for ii in range(NS):
        vr, vi = orig[ii]
        for jj in range(ii):
            ur, ui = orig[jj]
            nr = jnp.sum(ur * vr + ui * vi, axis=1, keepdims=True)
            ni = jnp.sum(ur * vi - ui * vr, axis=1, keepdims=True)
            cr = nr / den[jj]
            ci = ni / den[jj]
            vr = vr - (cr * ur - ci * ui)
            vi = vi - (cr * ui + ci * ur)
        inv = 1.0 / jnp.sqrt(jnp.sum(vr * vr + vi * vi, axis=1, keepdims=True))
        out.append((vr * inv, vi * inv))
    return (jnp.stack([o[0] for o in out], axis=2),
            jnp.stack([o[1] for o in out], axis=2))


def _logdet_hpd4(Mr, Mi):
    # logdet of Hermitian PD 4x4 batch via unrolled Cholesky. Returns (B,).
    L = [[None] * NS for _ in range(NS)]
    ld = 0.0
    for k in range(NS):
        d = Mr[:, k, k]
        for j in range(k):
            lr, li = L[k][j]
            d = d - (lr * lr + li * li)
        ld = ld + jnp.log(d)
        inv = 1.0 / jnp.sqrt(d)
        for i2 in range(k + 1, NS):
            pr = Mr[:, i2, k]
            pi = Mi[:, i2, k]
            for j in range(k):
                ar, ai = L[i2][j]
                br, bi = L[k][j]
                pr = pr - (ar * br + ai * bi)
                pi = pi - (ai * br - ar * bi)
            L[i2][k] = (pr * inv, pi * inv)
    return ld


# ---------------- per-stage computation (runs on one core's shard) ------


def _side_common(y, hs, gru_p, lin_p, mlp_w_p):
    x = _ri(y)
    hs = _gru(x, hs, gru_p)
    hlin = hs @ lin_p['w'] + lin_p['b']          # (NS, B, 128)
    w0 = _mlp_w(hs, mlp_w_p)                     # (B, NRF, NS)
    flat = jnp.moveaxis(hlin, 0, -1).reshape(hlin.shape[1], 128 * NS)
    return hs, w0, flat


def _stage(p, Hr, Hi, nbr, nbi, nar, nai, hs_a, hs_b,
           Far, Fai, Fbr, Fbi, FWar, FWai, ld_acc):
    H = (Hr, Hi)
    nstd = p['nstd']
    # ---- A transmits to B ----
    ybf = _cmatmul(H, (FWar, FWai))
    ybf = (ybf[0] + nbr * nstd, ybf[1] + nbi * nstd)
    yb = _cmatmul(_cT((Fbr, Fbi)), ybf)          # (B, NRF, NS)
    hs_b, Wb0, flat_b = _side_common(yb, hs_b, p['gru_b'], p['lin_b'],
                                     p['mlp_b_W'])
    Wb = _orthogonal(_cadd(Wb0, yb))
    Fbt = _unitmod(_mlp_f(flat_b, p['mlp_b_Ft'], NB), math.sqrt(NB))
    FWb = _colnorm(_cmatmul(Fbt, Wb), axis=1)
    Fb = _unitmod(_mlp_f(flat_b, p['mlp_b_Fr'], NB), math.sqrt(NB))
    # ---- B transmits to A ----
    yaf = _cmatmul(_cT(H), FWb)
    yaf = (yaf[0] + nar * nstd, yaf[1] + nai * nstd)
    ya = _cmatmul(_cT((Far, Fai)), yaf)
    hs_a, Wa0, flat_a = _side_common(ya, hs_a, p['gru_a'], p['lin_a'],
                                     p['mlp_a_W'])
    Wan = _orthogonal(_cadd(Wa0, ya))
    Fat = _unitmod(_mlp_f(flat_a, p['mlp_a_Ft'], NA), math.sqrt(NA))
    FWa = _colnorm(_cmatmul(Fat, Wan), axis=1)
    Fa = _unitmod(_mlp_f(flat_a, p['mlp_a_Fr'], NA), math.sqrt(NA))
    # ---- final beamformers ----
    Wb0f = _mlp_w(hs_b, p['mlp_b_final'])
    Wa0f = _mlp_w(hs_a, p['mlp_a_final'])
    Wbf = _colnorm(_cmatmul(Fb, _orthogonal(_cadd(Wb0f, yb))), axis=1)
    Waf = _colnorm(_cmatmul(Fa, _orthogonal(_cadd(Wa0f, ya))), axis=1)
    # ---- utility ----
    G = _cmatmul(_cmatmul(_cT(Wbf), H), Waf)     # (B, NS, NS)
    GH = _cT(G)
    Mr = G[0] @ GH[0] - G[1] @ GH[1] + jnp.eye(NS, dtype=jnp.float32)
    Mi = G[0] @ GH[1] + G[1] @ GH[0]
    ld_sum = ld_acc + jnp.sum(_logdet_hpd4(Mr, Mi))
    return (hs_a, hs_b, Fa[0], Fa[1], Fb[0], Fb[1], FWa[0], FWa[1],
            Waf[0], Waf[1], Wbf[0], Wbf[1], ld_sum)


def _forward8(p, Hr, Hi, nbr, nbi, nar, nai,
              Far, Fai, Fbr, Fbi, FWar, FWai):
    """All NSTAGES stages fused in one executable.

    nbr/nbi/nar/nai: (NSTAGES, B, n, NS) per-core noise slices.
    Far..FWai: (B, ...) initial (replicated) beamformer state.
    """
    hs_a = jnp.ones((NS, B, HSZ), jnp.float32)
    hs_b = jnp.ones((NS, B, HSZ), jnp.float32)
    ld_acc = jnp.float32(0.0)
    out = None
    for i in range(NSTAGES):
        (hs_a, hs_b, Far, Fai, Fbr, Fbi, FWar, FWai,
         Wafr, Wafi, Wbfr, Wbfi, ld_acc) = _stage(
            p, Hr, Hi, nbr[i], nbi[i], nar[i], nai[i],
            hs_a, hs_b, Far, Fai, Fbr, Fbi, FWar, FWai, ld_acc)
        out = (Wafr, Wafi, Wbfr, Wbfi)
    return out + (ld_acc,)


_PMAP_CACHE = {}


def _accel_devices():
    for plat in ('neuron', 'axon'):
        try:
            d = jax.devices(plat)
            if len(d) >= NCORES:
                return d[:NCORES]
        except RuntimeError:
            continue
    return jax.devices()[:NCORES]


def _get_fwd_pmap(devs):
    key = tuple(id(d) for d in devs)
    if key not in _PMAP_CACHE:
        _PMAP_CACHE[key] = jax.pmap(_forward8, axis_name='c', devices=devs)
    return _PMAP_CACHE[key]


def _np_leaves(d):
    if isinstance(d, dict):
        return {k: _np_leaves(v) for k, v in d.items()}
    return np.asarray(d)


def kernel(channel_re, channel_im, sigma2, noise_a_re, noise_a_im,
           noise_b_re, noise_b_im, params):
    params = _np_leaves(params)
    channel_re = np.asarray(channel_re)
    channel_im = np.asarray(channel_im)
    sigma2 = np.asarray(sigma2)

    nstd = float(np.sqrt(sigma2[0] / 2.0))

    # ---- host-side init (tiny, unbatched) ----
    def unitmod_np(re, im, scale):
        a = scale * np.sqrt(re * re + im * im)
        return (re / a).astype(np.float32), (im / a).astype(np.float32)

    Fbr0, Fbi0 = unitmod_np(np.asarray(params['init_Fb_re']),
                            np.asarray(params['init_Fb_im']), math.sqrt(NB))
    Far0, Fai0 = unitmod_np(np.asarray(params['init_Fa_re']),
                            np.asarray(params['init_Fa_im']), math.sqrt(NA))
    War, Wai = np.asarray(params['init_Wa_re']), np.asarray(params['init_Wa_im'])
    FWar0 = Far0 @ War - Fai0 @ Wai
    FWai0 = Far0 @ Wai + Fai0 @ War
    nrm = np.sqrt((FWar0 * FWar0 + FWai0 * FWai0).sum(axis=0, keepdims=True))
    FWar0 = (FWar0 / nrm).astype(np.float32)
    FWai0 = (FWai0 / nrm).astype(np.float32)

    # ---- shard data: (NCORES, B, ...) ----
    def shard(x):
        x = np.asarray(x)
        return x.reshape((NCORES, B) + x.shape[1:])

    devs = _accel_devices()
    fwd_pmap = _get_fwd_pmap(devs)

    def dshard(x):
        x = shard(x)
        return jax.device_put_sharded([x[i] for i in range(NCORES)], devs)

    def dshard_noise(x):
        # (NSTAGES, BSZ, n, NS) -> (NCORES, NSTAGES, B, n, NS)
        x = np.asarray(x)
        x = x.reshape(NSTAGES, NCORES, B, *x.shape[2:]).swapaxes(0, 1)
        return jax.device_put_sharded([np.ascontiguousarray(x[i])
                                       for i in range(NCORES)], devs)

    Hr = dshard(channel_re)
    Hi = dshard(channel_im)
    nb_r = dshard_noise(noise_b_re)
    nb_i = dshard_noise(noise_b_im)
    na_r = dshard_noise(noise_a_re)
    na_i = dshard_noise(noise_a_im)

    # ---- replicated params ----
    prep = {
        'gru_a': params['gru_a'], 'gru_b': params['gru_b'],
        'mlp_a_W': params['mlp_a_W'], 'mlp_b_W': params['mlp_b_W'],
        'mlp_a_final': params['mlp_a_final'],
        'mlp_b_final': params['mlp_b_final'],
        'mlp_a_Fr': params['mlp_a_Fr'], 'mlp_b_Fr': params['mlp_b_Fr'],
        'mlp_a_Ft': params['mlp_a_Ft'], 'mlp_b_Ft': params['mlp_b_Ft'],
        'lin_a': params['lin_a'], 'lin_b': params['lin_b'],
        'nstd': np.float32(nstd),
    }
    prep = jax.device_put_replicated(prep, devs)

    # ---- initial carry ----
    def drepl(a):  # replicate a small per-shard array to all cores
        return jax.device_put_sharded(
            [np.broadcast_to(a, (B,) + a.shape).copy() for _ in range(NCORES)],
            devs)

    Wafr, Wafi, Wbfr, Wbfi, ld = fwd_pmap(
        prep, Hr, Hi, nb_r, nb_i, na_r, na_i,
        drepl(Far0), drepl(Fai0), drepl(Fbr0), drepl(Fbi0),
        drepl(FWar0), drepl(FWai0))
    Waf = (Wafr, Wafi)
    Wbf = (Wbfr, Wbfi)

    ld_total = np.sum(np.asarray(ld), dtype=np.float64)
    Wa_final = (np.asarray(Waf[0]) + 1j * np.asarray(Waf[1])).astype(
        np.complex64).reshape(BSZ, NA, NS)
    Wb_final = (np.asarray(Wbf[0]) + 1j * np.asarray(Wbf[1])).astype(
        np.complex64).reshape(BSZ, NB, NS)
    loss = np.float32(-ld_total / BSZ)
    return Wa_final, Wb_final, loss


# revision 22
# speedup vs baseline: 626.8137x; 163.9410x over previous
"""Data-parallel Trainium kernel for nn_ActiveSensingFramework.

Strategy: pure data parallel over 8 NeuronCores (bsz 1024 -> 8 x 128).
The forward pass is rewritten in real arithmetic (re/im pairs, no complex
dtype, no slogdet -- unrolled 4x4 Cholesky) so it lowers cleanly through
neuronx-cc, and executed with jax.pmap on the 8 cores. Parameters are
replicated once; only the scalar per-stage logdet sums are reduced on host.
"""

import math
from functools import partial

import numpy as np
import jax
import jax.numpy as jnp

HSZ, NSTAGES, NS, NA, NB, NRF, BSZ = 512, 8, 4, 64, 64, 8, 1024
NCORES = 8
B = BSZ // NCORES

# ---------------- complex helpers on (re, im) tuples ----------------


def _bmm(x, y):
    # Batched small matmul as broadcast-mul + reduce. XLA-on-neuron lowers
    # per-batch-element dot_general into ~1 tiny PE matmul + weight reload
    # per element (126K matmul instructions, 2.3% MFU, HAM half-clock);
    # mul+reduce lowers onto the idle vector engine instead.
    return jnp.sum(x[..., :, :, None] * y[..., None, :, :], axis=-2)


def _cmatmul(a, b):
    ar, ai = a
    br, bi = b
    return _bmm(ar, br) - _bmm(ai, bi), _bmm(ar, bi) + _bmm(ai, br)


def _cT(a):  # conj-transpose of last two dims
    ar, ai = a
    return jnp.swapaxes(ar, -1, -2), -jnp.swapaxes(ai, -1, -2)


def _cadd(a, b):
    return a[0] + b[0], a[1] + b[1]


def _unitmod(a, scale):
    # a / (scale * |a|)
    ar, ai = a
    inv = 1.0 / (scale * jnp.sqrt(ar * ar + ai * ai))
    return ar * inv, ai * inv


def _colnorm(a, axis):
    ar, ai = a
    inv = 1.0 / jnp.sqrt(jnp.sum(ar * ar + ai * ai, axis=axis, keepdims=True))
    return ar * inv, ai * inv


def _ri(y):
    # (B, n, ns) complex pair -> (ns, B, 2n) interleaved re/im
    yr = jnp.moveaxis(y[0], 2, 0)
    yi = jnp.moveaxis(y[1], 2, 0)
    s = jnp.stack([yr, yi], axis=-1)  # (ns, B, n, 2)
    return s.reshape(s.shape[0], s.shape[1], -1)


def _mlp_w(x, p):
    # x: (NS, B, HSZ) -> complex (B, NRF, NS)
    o = jax.nn.relu(x @ p['w1'] + p['b1']) @ p['w2'] + p['b2']
    re = jnp.moveaxis(o[..., :NRF], 0, -1)      # (B, NRF, NS)
    im = jnp.moveaxis(o[..., NRF:2 * NRF], 0, -1)
    return re, im


def _mlp_f(x, p, n):
    # x: (B, 512) -> complex (B, n, NRF)
    o = jax.nn.relu(x @ p['w1'] + p['b1']) @ p['w2'] + p['b2']
    m = n * NRF
    re = o[..., :m].reshape(x.shape[0], n, NRF)
    im = o[..., m:2 * m].reshape(x.shape[0], n, NRF)
    return re, im


def _gru(x, h, p):
    gi = x @ p['wih'] + p['bih']
    gh = h @ p['whh'] + p['bhh']
    ir, iz, inn = jnp.split(gi, 3, axis=-1)
    hr, hz, hn = jnp.split(gh, 3, axis=-1)
    r = jax.nn.sigmoid(ir + hr)
    z = jax.nn.sigmoid(iz + hz)
    n = jnp.tanh(inn + r * hn)
    return (1.0 - z) * n + z * h


def _orthogonal(x):
    # x: complex pair, (B, n, NS). Projections onto ORIGINAL columns.
    xr, xi = x
    orig = [(xr[:, :, k], xi[:, :, k]) for k in range(NS)]
    den = [jnp.sum(o[0] * o[0] + o[1] * o[1], axis=1, keepdims=True)
           for o in orig]
    out = []
    for ii in range(NS):
        vr, vi = orig[ii]
        for jj in range(ii):
            ur, ui = orig[jj]
            nr = jnp.sum(ur * vr + ui * vi, axis=1, keepdims=True)
            ni = jnp.sum(ur * vi - ui * vr, axis=1, keepdims=True)
            cr = nr / den[jj]
            ci = ni / den[jj]
            vr = vr - (cr * ur - ci * ui)
            vi = vi - (cr * ui + ci * ur)
        inv = 1.0 / jnp.sqrt(jnp.sum(vr * vr + vi * vi, axis=1, keepdims=True))
        out.append((vr * inv, vi * inv))
    return (jnp.stack([o[0] for o in out], axis=2),
            jnp.stack([o[1] for o in out], axis=2))


def _logdet_hpd4(Mr, Mi):
    # logdet of Hermitian PD 4x4 batch via unrolled Cholesky. Returns (B,).
    L = [[None] * NS for _ in range(NS)]
    ld = 0.0
    for k in range(NS):
        d = Mr[:, k, k]
        for j in range(k):
            lr, li = L[k][j]
            d = d - (lr * lr + li * li)
        ld = ld + jnp.log(d)
        inv = 1.0 / jnp.sqrt(d)
        for i2 in range(k + 1, NS):
            pr = Mr[:, i2, k]
            pi = Mi[:, i2, k]
            for j in range(k):
                ar, ai = L[i2][j]
                br, bi = L[k][j]
                pr = pr - (ar * br + ai * bi)
                pi = pi - (ai * br - ar * bi)
            L[i2][k] = (pr * inv, pi * inv)
    return ld


# ---------------- per-stage computation (runs on one core's shard) ------


def _side_common(y, hs, gru_p, lin_p, mlp_w_p):
    x = _ri(y)
    hs = _gru(x, hs, gru_p)
    hlin = hs @ lin_p['w'] + lin_p['b']          # (NS, B, 128)
    w0 = _mlp_w(hs, mlp_w_p)                     # (B, NRF, NS)
    flat = jnp.moveaxis(hlin, 0, -1).reshape(hlin.shape[1], 128 * NS)
    return hs, w0, flat


def _stage(p, Hr, Hi, nbr, nbi, nar, nai, hs_a, hs_b,
           Far, Fai, Fbr, Fbi, FWar, FWai, ld_acc):
    H = (Hr, Hi)
    nstd = p['nstd']
    # ---- A transmits to B ----
    ybf = _cmatmul(H, (FWar, FWai))
    ybf = (ybf[0] + nbr * nstd, ybf[1] + nbi * nstd)
    yb = _cmatmul(_cT((Fbr, Fbi)), ybf)          # (B, NRF, NS)
    hs_b, Wb0, flat_b = _side_common(yb, hs_b, p['gru_b'], p['lin_b'],
                                     p['mlp_b_W'])
    Wb = _orthogonal(_cadd(Wb0, yb))
    Fbt = _unitmod(_mlp_f(flat_b, p['mlp_b_Ft'], NB), math.sqrt(NB))
    FWb = _colnorm(_cmatmul(Fbt, Wb), axis=1)
    Fb = _unitmod(_mlp_f(flat_b, p['mlp_b_Fr'], NB), math.sqrt(NB))
    # ---- B transmits to A ----
    yaf = _cmatmul(_cT(H), FWb)
    yaf = (yaf[0] + nar * nstd, yaf[1] + nai * nstd)
    ya = _cmatmul(_cT((Far, Fai)), yaf)
    hs_a, Wa0, flat_a = _side_common(ya, hs_a, p['gru_a'], p['lin_a'],
                                     p['mlp_a_W'])
    Wan = _orthogonal(_cadd(Wa0, ya))
    Fat = _unitmod(_mlp_f(flat_a, p['mlp_a_Ft'], NA), math.sqrt(NA))
    FWa = _colnorm(_cmatmul(Fat, Wan), axis=1)
    Fa = _unitmod(_mlp_f(flat_a, p['mlp_a_Fr'], NA), math.sqrt(NA))
    # ---- final beamformers ----
    Wb0f = _mlp_w(hs_b, p['mlp_b_final'])
    Wa0f = _mlp_w(hs_a, p['mlp_a_final'])
    Wbf = _colnorm(_cmatmul(Fb, _orthogonal(_cadd(Wb0f, yb))), axis=1)
    Waf = _colnorm(_cmatmul(Fa, _orthogonal(_cadd(Wa0f, ya))), axis=1)
    # ---- utility ----
    G = _cmatmul(_cmatmul(_cT(Wbf), H), Waf)     # (B, NS, NS)
    GH = _cT(G)
    Mr = _bmm(G[0], GH[0]) - _bmm(G[1], GH[1]) + jnp.eye(NS, dtype=jnp.float32)
    Mi = _bmm(G[0], GH[1]) + _bmm(G[1], GH[0])
    ld_sum = ld_acc + jnp.sum(_logdet_hpd4(Mr, Mi))
    return (hs_a, hs_b, Fa[0], Fa[1], Fb[0], Fb[1], FWa[0], FWa[1],
            Waf[0], Waf[1], Wbf[0], Wbf[1], ld_sum)


def _forward8(p, Hr, Hi, nbr, nbi, nar, nai,
              Far, Fai, Fbr, Fbi, FWar, FWai):
    """All NSTAGES stages fused in one executable.

    nbr/nbi/nar/nai: (NSTAGES, B, n, NS) per-core noise slices (fp16).
    Far..FWai: (B, ...) initial (replicated) beamformer state.
    """
    f32 = jnp.float32
    Hr = Hr.astype(f32)
    Hi = Hi.astype(f32)
    nbr = nbr.astype(f32)
    nbi = nbi.astype(f32)
    nar = nar.astype(f32)
    nai = nai.astype(f32)
    hs_a = jnp.ones((NS, B, HSZ), jnp.float32)
    hs_b = jnp.ones((NS, B, HSZ), jnp.float32)
    ld_acc = jnp.float32(0.0)
    out = None
    for i in range(NSTAGES):
        (hs_a, hs_b, Far, Fai, Fbr, Fbi, FWar, FWai,
         Wafr, Wafi, Wbfr, Wbfi, ld_acc) = _stage(
            p, Hr, Hi, nbr[i], nbi[i], nar[i], nai[i],
            hs_a, hs_b, Far, Fai, Fbr, Fbi, FWar, FWai, ld_acc)
        out = (Wafr, Wafi, Wbfr, Wbfi)
    return out + (ld_acc,)


_PMAP_CACHE = {}


def _accel_devices():
    for plat in ('neuron', 'axon'):
        try:
            d = jax.devices(plat)
            if len(d) >= NCORES:
                return d[:NCORES]
        except RuntimeError:
            continue
    return jax.devices()[:NCORES]


def _get_fwd_pmap(devs):
    key = tuple(id(d) for d in devs)
    if key not in _PMAP_CACHE:
        _PMAP_CACHE[key] = jax.pmap(_forward8, axis_name='c', devices=devs)
    return _PMAP_CACHE[key]


def _np_leaves(d):
    if isinstance(d, dict):
        return {k: _np_leaves(v) for k, v in d.items()}
    return np.asarray(d)


def prepare(channel_re, channel_im, sigma2, noise_a_re, noise_a_im,
            noise_b_re, noise_b_im, params):
    """Stage all inputs onto the 8 cores; returns (pmap_fn, device_args)."""
    params = _np_leaves(params)
    channel_re = np.asarray(channel_re)
    channel_im = np.asarray(channel_im)
    sigma2 = np.asarray(sigma2)

    nstd = float(np.sqrt(sigma2[0] / 2.0))

    # ---- host-side init (tiny, unbatched) ----
    def unitmod_np(re, im, scale):
        a = scale * np.sqrt(re * re + im * im)
        return (re / a).astype(np.float32), (im / a).astype(np.float32)

    Fbr0, Fbi0 = unitmod_np(np.asarray(params['init_Fb_re']),
                            np.asarray(params['init_Fb_im']), math.sqrt(NB))
    Far0, Fai0 = unitmod_np(np.asarray(params['init_Fa_re']),
                            np.asarray(params['init_Fa_im']), math.sqrt(NA))
    War, Wai = np.asarray(params['init_Wa_re']), np.asarray(params['init_Wa_im'])
    FWar0 = Far0 @ War - Fai0 @ Wai
    FWai0 = Far0 @ Wai + Fai0 @ War
    nrm = np.sqrt((FWar0 * FWar0 + FWai0 * FWai0).sum(axis=0, keepdims=True))
    FWar0 = (FWar0 / nrm).astype(np.float32)
    FWai0 = (FWai0 / nrm).astype(np.float32)

    # ---- shard data: (NCORES, B, ...) ----
    def shard(x):
        x = np.asarray(x)
        return x.reshape((NCORES, B) + x.shape[1:])

    devs = _accel_devices()
    fwd_pmap = _get_fwd_pmap(devs)

    def dshard(x):
        # fp32 on the wire: the pipeline (Gram-Schmidt + 8 recurrent
        # stages) is chaotic -- fp16-quantized inputs shift the output by
        # ~0.2 rel, so inputs must stay exact even though the tunnel is slow.
        x = shard(x).astype(np.float32)
        return jax.device_put_sharded([x[i] for i in range(NCORES)], devs)

    def dshard_noise(x):
        # (NSTAGES, BSZ, n, NS) -> (NCORES, NSTAGES, B, n, NS)
        x = np.asarray(x).astype(np.float32)
        x = x.reshape(NSTAGES, NCORES, B, *x.shape[2:]).swapaxes(0, 1)
        return jax.device_put_sharded([np.ascontiguousarray(x[i])
                                       for i in range(NCORES)], devs)

    Hr = dshard(channel_re)
    Hi = dshard(channel_im)
    nb_r = dshard_noise(noise_b_re)
    nb_i = dshard_noise(noise_b_im)
    na_r = dshard_noise(noise_a_re)
    na_i = dshard_noise(noise_a_im)

    # ---- replicated params ----
    prep = {
        'gru_a': params['gru_a'], 'gru_b': params['gru_b'],
        'mlp_a_W': params['mlp_a_W'], 'mlp_b_W': params['mlp_b_W'],
        'mlp_a_final': params['mlp_a_final'],
        'mlp_b_final': params['mlp_b_final'],
        'mlp_a_Fr': params['mlp_a_Fr'], 'mlp_b_Fr': params['mlp_b_Fr'],
        'mlp_a_Ft': params['mlp_a_Ft'], 'mlp_b_Ft': params['mlp_b_Ft'],
        'lin_a': params['lin_a'], 'lin_b': params['lin_b'],
        'nstd': np.float32(nstd),
    }
    prep = jax.device_put_replicated(prep, devs)

    # ---- initial carry ----
    def drepl(a):  # replicate a small per-shard array to all cores
        return jax.device_put_sharded(
            [np.broadcast_to(a, (B,) + a.shape).copy() for _ in range(NCORES)],
            devs)

    dargs = (prep, Hr, Hi, nb_r, nb_i, na_r, na_i,
             drepl(Far0), drepl(Fai0), drepl(Fbr0), drepl(Fbi0),
             drepl(FWar0), drepl(FWai0))
    return fwd_pmap, dargs


def run(fwd_pmap, dargs):
    """Execute the fused 8-stage pass and gather the full outputs."""
    Wafr, Wafi, Wbfr, Wbfi, ld = fwd_pmap(*dargs)
    ld_total = np.sum(np.asarray(ld), dtype=np.float64)
    Wa_final = (np.asarray(Wafr) + 1j * np.asarray(Wafi)).astype(
        np.complex64).reshape(BSZ, NA, NS)
    Wb_final = (np.asarray(Wbfr) + 1j * np.asarray(Wbfi)).astype(
        np.complex64).reshape(BSZ, NB, NS)
    loss = np.float32(-ld_total / BSZ)
    return Wa_final, Wb_final, loss


def kernel(channel_re, channel_im, sigma2, noise_a_re, noise_a_im,
           noise_b_re, noise_b_im, params):
    fwd_pmap, dargs = prepare(channel_re, channel_im, sigma2, noise_a_re,
                              noise_a_im, noise_b_re, noise_b_im, params)
    return run(fwd_pmap, dargs)


# revision 23
# speedup vs baseline: 1371.8300x; 2.1886x over previous
"""Data-parallel Trainium kernel for nn_ActiveSensingFramework.

Strategy: pure data parallel over 8 NeuronCores (bsz 1024 -> 8 x 128).
The forward pass is rewritten in real arithmetic (re/im pairs, no complex
dtype, no slogdet -- unrolled 4x4 Cholesky) so it lowers cleanly through
neuronx-cc, and executed with jax.pmap on the 8 cores. Parameters are
replicated once; only the scalar per-stage logdet sums are reduced on host.
"""

import math
from functools import partial

import numpy as np
import jax
import jax.numpy as jnp

HSZ, NSTAGES, NS, NA, NB, NRF, BSZ = 512, 8, 4, 64, 64, 8, 1024
NCORES = 8
B = BSZ // NCORES

# ---------------- complex helpers on (re, im) tuples ----------------


def _bmm(x, y):
    # Batched small matmul as broadcast-mul + reduce. XLA-on-neuron lowers
    # per-batch-element dot_general into ~1 tiny PE matmul + weight reload
    # per element (126K matmul instructions, 2.3% MFU, HAM half-clock);
    # mul+reduce lowers onto the idle vector engine instead.
    return jnp.sum(x[..., :, :, None] * y[..., None, :, :], axis=-2)


def _cmatmul(a, b):
    # Complex batched matmul as ONE real batched matmul of double K/N:
    # [ar | -ai] @ [[br, bi], [bi, -br]] = [Re | Im]. Same total flops as
    # the 4-real-matmul form, but neuronx-cc emits one tiny PE matmul +
    # weight load per batch element per dot_general, so fusing 4 dots into
    # 1 quarters the instruction count and halves the LDWEIGHTS rows.
    ar, ai = a
    br, bi = b
    A = jnp.concatenate([ar, -ai], axis=-1)
    Bc = jnp.concatenate([jnp.concatenate([br, bi], axis=-1),
                          jnp.concatenate([bi, -br], axis=-1)], axis=-2)
    o = A @ Bc
    n = br.shape[-1]
    return o[..., :n], o[..., n:]


def _cT(a):  # conj-transpose of last two dims
    ar, ai = a
    return jnp.swapaxes(ar, -1, -2), -jnp.swapaxes(ai, -1, -2)


def _cadd(a, b):
    return a[0] + b[0], a[1] + b[1]


def _unitmod(a, scale):
    # a / (scale * |a|)
    ar, ai = a
    inv = 1.0 / (scale * jnp.sqrt(ar * ar + ai * ai))
    return ar * inv, ai * inv


def _colnorm(a, axis):
    ar, ai = a
    inv = 1.0 / jnp.sqrt(jnp.sum(ar * ar + ai * ai, axis=axis, keepdims=True))
    return ar * inv, ai * inv


def _ri(y):
    # (B, n, ns) complex pair -> (ns, B, 2n) interleaved re/im
    yr = jnp.moveaxis(y[0], 2, 0)
    yi = jnp.moveaxis(y[1], 2, 0)
    s = jnp.stack([yr, yi], axis=-1)  # (ns, B, n, 2)
    return s.reshape(s.shape[0], s.shape[1], -1)


def _mlp_w(x, p):
    # x: (NS, B, HSZ) -> complex (B, NRF, NS)
    o = jax.nn.relu(x @ p['w1'] + p['b1']) @ p['w2'] + p['b2']
    re = jnp.moveaxis(o[..., :NRF], 0, -1)      # (B, NRF, NS)
    im = jnp.moveaxis(o[..., NRF:2 * NRF], 0, -1)
    return re, im


def _mlp_f(x, p, n):
    # x: (B, 512) -> complex (B, n, NRF)
    o = jax.nn.relu(x @ p['w1'] + p['b1']) @ p['w2'] + p['b2']
    m = n * NRF
    re = o[..., :m].reshape(x.shape[0], n, NRF)
    im = o[..., m:2 * m].reshape(x.shape[0], n, NRF)
    return re, im


def _gru(x, h, p):
    gi = x @ p['wih'] + p['bih']
    gh = h @ p['whh'] + p['bhh']
    ir, iz, inn = jnp.split(gi, 3, axis=-1)
    hr, hz, hn = jnp.split(gh, 3, axis=-1)
    r = jax.nn.sigmoid(ir + hr)
    z = jax.nn.sigmoid(iz + hz)
    n = jnp.tanh(inn + r * hn)
    return (1.0 - z) * n + z * h


def _orthogonal(x):
    # x: complex pair, (B, n, NS). Projections onto ORIGINAL columns.
    xr, xi = x
    orig = [(xr[:, :, k], xi[:, :, k]) for k in range(NS)]
    den = [jnp.sum(o[0] * o[0] + o[1] * o[1], axis=1, keepdims=True)
           for o in orig]
    out = []
    for ii in range(NS):
        vr, vi = orig[ii]
        for jj in range(ii):
            ur, ui = orig[jj]
            nr = jnp.sum(ur * vr + ui * vi, axis=1, keepdims=True)
            ni = jnp.sum(ur * vi - ui * vr, axis=1, keepdims=True)
            cr = nr / den[jj]
            ci = ni / den[jj]
            vr = vr - (cr * ur - ci * ui)
            vi = vi - (cr * ui + ci * ur)
        inv = 1.0 / jnp.sqrt(jnp.sum(vr * vr + vi * vi, axis=1, keepdims=True))
        out.append((vr * inv, vi * inv))
    return (jnp.stack([o[0] for o in out], axis=2),
            jnp.stack([o[1] for o in out], axis=2))


def _logdet_hpd4(Mr, Mi):
    # logdet of Hermitian PD 4x4 batch via unrolled Cholesky. Returns (B,).
    L = [[None] * NS for _ in range(NS)]
    ld = 0.0
    for k in range(NS):
        d = Mr[:, k, k]
        for j in range(k):
            lr, li = L[k][j]
            d = d - (lr * lr + li * li)
        ld = ld + jnp.log(d)
        inv = 1.0 / jnp.sqrt(d)
        for i2 in range(k + 1, NS):
            pr = Mr[:, i2, k]
            pi = Mi[:, i2, k]
            for j in range(k):
                ar, ai = L[i2][j]
                br, bi = L[k][j]
                pr = pr - (ar * br + ai * bi)
                pi = pi - (ai * br - ar * bi)
            L[i2][k] = (pr * inv, pi * inv)
    return ld


# ---------------- per-stage computation (runs on one core's shard) ------


def _side_common(y, hs, gru_p, lin_p, mlp_w_p):
    x = _ri(y)
    hs = _gru(x, hs, gru_p)
    hlin = hs @ lin_p['w'] + lin_p['b']          # (NS, B, 128)
    w0 = _mlp_w(hs, mlp_w_p)                     # (B, NRF, NS)
    flat = jnp.moveaxis(hlin, 0, -1).reshape(hlin.shape[1], 128 * NS)
    return hs, w0, flat


def _stage(p, Hr, Hi, nbr, nbi, nar, nai, hs_a, hs_b,
           Far, Fai, Fbr, Fbi, FWar, FWai, ld_acc):
    H = (Hr, Hi)
    nstd = p['nstd']
    # ---- A transmits to B ----
    ybf = _cmatmul(H, (FWar, FWai))
    ybf = (ybf[0] + nbr * nstd, ybf[1] + nbi * nstd)
    yb = _cmatmul(_cT((Fbr, Fbi)), ybf)          # (B, NRF, NS)
    hs_b, Wb0, flat_b = _side_common(yb, hs_b, p['gru_b'], p['lin_b'],
                                     p['mlp_b_W'])
    Wb = _orthogonal(_cadd(Wb0, yb))
    Fbt = _unitmod(_mlp_f(flat_b, p['mlp_b_Ft'], NB), math.sqrt(NB))
    FWb = _colnorm(_cmatmul(Fbt, Wb), axis=1)
    Fb = _unitmod(_mlp_f(flat_b, p['mlp_b_Fr'], NB), math.sqrt(NB))
    # ---- B transmits to A ----
    yaf = _cmatmul(_cT(H), FWb)
    yaf = (yaf[0] + nar * nstd, yaf[1] + nai * nstd)
    ya = _cmatmul(_cT((Far, Fai)), yaf)
    hs_a, Wa0, flat_a = _side_common(ya, hs_a, p['gru_a'], p['lin_a'],
                                     p['mlp_a_W'])
    Wan = _orthogonal(_cadd(Wa0, ya))
    Fat = _unitmod(_mlp_f(flat_a, p['mlp_a_Ft'], NA), math.sqrt(NA))
    FWa = _colnorm(_cmatmul(Fat, Wan), axis=1)
    Fa = _unitmod(_mlp_f(flat_a, p['mlp_a_Fr'], NA), math.sqrt(NA))
    # ---- final beamformers ----
    Wb0f = _mlp_w(hs_b, p['mlp_b_final'])
    Wa0f = _mlp_w(hs_a, p['mlp_a_final'])
    Wbf = _colnorm(_cmatmul(Fb, _orthogonal(_cadd(Wb0f, yb))), axis=1)
    Waf = _colnorm(_cmatmul(Fa, _orthogonal(_cadd(Wa0f, ya))), axis=1)
    # ---- utility ----
    G = _cmatmul(_cmatmul(_cT(Wbf), H), Waf)     # (B, NS, NS)
    GH = _cT(G)
    Mr = _bmm(G[0], GH[0]) - _bmm(G[1], GH[1]) + jnp.eye(NS, dtype=jnp.float32)
    Mi = _bmm(G[0], GH[1]) + _bmm(G[1], GH[0])
    ld_sum = ld_acc + jnp.sum(_logdet_hpd4(Mr, Mi))
    return (hs_a, hs_b, Fa[0], Fa[1], Fb[0], Fb[1], FWa[0], FWa[1],
            Waf[0], Waf[1], Wbf[0], Wbf[1], ld_sum)


def _forward8(p, Hr, Hi, nbr, nbi, nar, nai,
              Far, Fai, Fbr, Fbi, FWar, FWai):
    """All NSTAGES stages fused in one executable.

    nbr/nbi/nar/nai: (NSTAGES, B, n, NS) per-core noise slices (fp16).
    Far..FWai: (B, ...) initial (replicated) beamformer state.
    """
    f32 = jnp.float32
    Hr = Hr.astype(f32)
    Hi = Hi.astype(f32)
    nbr = nbr.astype(f32)
    nbi = nbi.astype(f32)
    nar = nar.astype(f32)
    nai = nai.astype(f32)
    hs_a = jnp.ones((NS, B, HSZ), jnp.float32)
    hs_b = jnp.ones((NS, B, HSZ), jnp.float32)
    ld_acc = jnp.float32(0.0)
    out = None
    for i in range(NSTAGES):
        (hs_a, hs_b, Far, Fai, Fbr, Fbi, FWar, FWai,
         Wafr, Wafi, Wbfr, Wbfi, ld_acc) = _stage(
            p, Hr, Hi, nbr[i], nbi[i], nar[i], nai[i],
            hs_a, hs_b, Far, Fai, Fbr, Fbi, FWar, FWai, ld_acc)
        out = (Wafr, Wafi, Wbfr, Wbfi)
    return out + (ld_acc,)


_PMAP_CACHE = {}


def _accel_devices():
    for plat in ('neuron', 'axon'):
        try:
            d = jax.devices(plat)
            if len(d) >= NCORES:
                return d[:NCORES]
        except RuntimeError:
            continue
    return jax.devices()[:NCORES]


def _get_fwd_pmap(devs):
    key = tuple(id(d) for d in devs)
    if key not in _PMAP_CACHE:
        _PMAP_CACHE[key] = jax.pmap(_forward8, axis_name='c', devices=devs)
    return _PMAP_CACHE[key]


def _np_leaves(d):
    if isinstance(d, dict):
        return {k: _np_leaves(v) for k, v in d.items()}
    return np.asarray(d)


def prepare(channel_re, channel_im, sigma2, noise_a_re, noise_a_im,
            noise_b_re, noise_b_im, params):
    """Stage all inputs onto the 8 cores; returns (pmap_fn, device_args)."""
    params = _np_leaves(params)
    channel_re = np.asarray(channel_re)
    channel_im = np.asarray(channel_im)
    sigma2 = np.asarray(sigma2)

    nstd = float(np.sqrt(sigma2[0] / 2.0))

    # ---- host-side init (tiny, unbatched) ----
    def unitmod_np(re, im, scale):
        a = scale * np.sqrt(re * re + im * im)
        return (re / a).astype(np.float32), (im / a).astype(np.float32)

    Fbr0, Fbi0 = unitmod_np(np.asarray(params['init_Fb_re']),
                            np.asarray(params['init_Fb_im']), math.sqrt(NB))
    Far0, Fai0 = unitmod_np(np.asarray(params['init_Fa_re']),
                            np.asarray(params['init_Fa_im']), math.sqrt(NA))
    War, Wai = np.asarray(params['init_Wa_re']), np.asarray(params['init_Wa_im'])
    FWar0 = Far0 @ War - Fai0 @ Wai
    FWai0 = Far0 @ Wai + Fai0 @ War
    nrm = np.sqrt((FWar0 * FWar0 + FWai0 * FWai0).sum(axis=0, keepdims=True))
    FWar0 = (FWar0 / nrm).astype(np.float32)
    FWai0 = (FWai0 / nrm).astype(np.float32)

    # ---- shard data: (NCORES, B, ...) ----
    def shard(x):
        x = np.asarray(x)
        return x.reshape((NCORES, B) + x.shape[1:])

    devs = _accel_devices()
    fwd_pmap = _get_fwd_pmap(devs)

    def dshard(x):
        # fp32 on the wire: the pipeline (Gram-Schmidt + 8 recurrent
        # stages) is chaotic -- fp16-quantized inputs shift the output by
        # ~0.2 rel, so inputs must stay exact even though the tunnel is slow.
        x = shard(x).astype(np.float32)
        return jax.device_put_sharded([x[i] for i in range(NCORES)], devs)

    def dshard_noise(x):
        # (NSTAGES, BSZ, n, NS) -> (NCORES, NSTAGES, B, n, NS)
        x = np.asarray(x).astype(np.float32)
        x = x.reshape(NSTAGES, NCORES, B, *x.shape[2:]).swapaxes(0, 1)
        return jax.device_put_sharded([np.ascontiguousarray(x[i])
                                       for i in range(NCORES)], devs)

    Hr = dshard(channel_re)
    Hi = dshard(channel_im)
    nb_r = dshard_noise(noise_b_re)
    nb_i = dshard_noise(noise_b_im)
    na_r = dshard_noise(noise_a_re)
    na_i = dshard_noise(noise_a_im)

    # ---- replicated params ----
    prep = {
        'gru_a': params['gru_a'], 'gru_b': params['gru_b'],
        'mlp_a_W': params['mlp_a_W'], 'mlp_b_W': params['mlp_b_W'],
        'mlp_a_final': params['mlp_a_final'],
        'mlp_b_final': params['mlp_b_final'],
        'mlp_a_Fr': params['mlp_a_Fr'], 'mlp_b_Fr': params['mlp_b_Fr'],
        'mlp_a_Ft': params['mlp_a_Ft'], 'mlp_b_Ft': params['mlp_b_Ft'],
        'lin_a': params['lin_a'], 'lin_b': params['lin_b'],
        'nstd': np.float32(nstd),
    }
    prep = jax.device_put_replicated(prep, devs)

    # ---- initial carry ----
    def drepl(a):  # replicate a small per-shard array to all cores
        return jax.device_put_sharded(
            [np.broadcast_to(a, (B,) + a.shape).copy() for _ in range(NCORES)],
            devs)

    dargs = (prep, Hr, Hi, nb_r, nb_i, na_r, na_i,
             drepl(Far0), drepl(Fai0), drepl(Fbr0), drepl(Fbi0),
             drepl(FWar0), drepl(FWai0))
    return fwd_pmap, dargs


def run(fwd_pmap, dargs):
    """Execute the fused 8-stage pass and gather the full outputs."""
    Wafr, Wafi, Wbfr, Wbfi, ld = fwd_pmap(*dargs)
    ld_total = np.sum(np.asarray(ld), dtype=np.float64)
    Wa_final = (np.asarray(Wafr) + 1j * np.asarray(Wafi)).astype(
        np.complex64).reshape(BSZ, NA, NS)
    Wb_final = (np.asarray(Wbfr) + 1j * np.asarray(Wbfi)).astype(
        np.complex64).reshape(BSZ, NB, NS)
    loss = np.float32(-ld_total / BSZ)
    return Wa_final, Wb_final, loss


def kernel(channel_re, channel_im, sigma2, noise_a_re, noise_a_im,
           noise_b_re, noise_b_im, params):
    fwd_pmap, dargs = prepare(channel_re, channel_im, sigma2, noise_a_re,
                              noise_a_im, noise_b_re, noise_b_im, params)
    return run(fwd_pmap, dargs)
